# revision 13
# baseline (speedup 1.0000x reference)
"""FINN Burgers solver (nn_FINN_Burger) as a Trainium2 Bass kernel.

The per-point MLP a = tanh(tanh(tanh(u W1) W2) W3) is a scalar function
F: R -> R of the cell value u alone.  F is smooth (max |F''| ~ 1.3), so a
128-knot piecewise-linear interpolant matches it to ~1e-3 -- far inside the
harness tolerance.  The kernel therefore:

  1. builds the table on device: one exact-MLP evaluation at the 128 knot
     positions (same matmul pipeline as the baseline kernel, one-time cost),
  2. per Euler step evaluates a = PWL(u) via a "two-hot" matmul:
        y    = u/h                      (PE ones-broadcast to 128 partitions)
        t1   = |y - c_p|                (ACT Abs, per-partition bias c_p)
        S    = relu(1 - t1)             (DVE, two bf16 4x-mode ops)
        a    = T^T @ S                  (PE matmul, table as weights)
     S holds exactly the two interpolation weights per point, so the matmul
     contraction over the 128 knot partitions IS the interpolation,
  3. computes the flux + Euler + mask update as  unw = P*a + Q*|a| + R  in a
     [128,17] 2-D layout, where P, Q, R depend only on u and are computed
     during the LUT phase (off the critical path):
        flux = D*lap + (dd*a + lap*|a|) / (2*DX),  dd = u_l - u_r,
        lap  = u_l + u_r - 2u   (relu(a) = (a+|a|)/2, min(a,0) = (a-|a|)/2)

The slab is processed as two 1088-point halves so the end-of-step reshape
DMAs of one half overlap the LUT phase of the other half / the next step.

Sharding: Nx=16384 split across 8 cores (2048 points each) with a 64-point
ghost zone per side; 15 steps need only a 15-point halo, so each core
integrates its 2176-point slab fully locally -- zero inter-core traffic.
Out-of-domain ghost points are zeroed each step via the mask (also the
Dirichlet boundary for cores 0 and 7).
"""

import dataclasses

import numpy as np

import concourse.bacc as bacc
import concourse.bass as bass
import concourse.mybir as mybir
from concourse import tile
from concourse.bass_utils import run_bass_kernel_spmd

F32 = mybir.dt.float32
F32R = mybir.dt.float32r
BF16 = mybir.dt.bfloat16
AF = mybir.ActivationFunctionType
OP = mybir.AluOpType

NX, H, NT = 16384, 512, 16
NCORES = 8
OWN = NX // NCORES          # 2048 points owned per core
P2, B2 = 128, 17            # 2-D layout: partition p holds points [17p,17p+17)
NP = P2 * B2                # 2176-point slab
GH = (NP - OWN) // 2        # 64-point ghost zone per side (need >= 15)
NSTEP = NT - 1
DX = 0.01
D_COEF = 0.01

K = 128                     # PWL knots
LO, HI = -5.5, 5.5
HSTEP = (HI - LO) / (K - 1)
HP = NP // 2                # 1088 points per half
# per-half point chunks (offsets relative to half start)
HCH = [(0, 512), (512, 512), (1024, 64)]


def _build_nc(nrep=1):
    nc = bacc.Bacc("TRN2", target_bir_lowering=False, debug=False)

    u0g = nc.dram_tensor("u0g", [1, NP + 2], F32, kind="ExternalInput")
    w1d = nc.dram_tensor("w1", [1, H], F32, kind="ExternalInput")
    w2d = nc.dram_tensor("w2", [H, H], F32, kind="ExternalInput")
    w3d = nc.dram_tensor("w3", [H, 1], F32, kind="ExternalInput")
    tbd = nc.dram_tensor("tb", [128, NT], F32, kind="ExternalInput")
    mkd = nc.dram_tensor("mask", [P2, B2], F32, kind="ExternalInput")
    knd = nc.dram_tensor("kn", [1, K], F32, kind="ExternalInput")
    bvd = nc.dram_tensor("biasv", [128, 1], F32, kind="ExternalInput")
    outd = nc.dram_tensor("out", [NT, OWN], F32, kind="ExternalOutput")

    with tile.TileContext(nc) as tc:
        with (
            tc.tile_pool(name="pers", bufs=1) as pers,
            tc.tile_pool(name="bld", bufs=1) as bld,
            tc.tile_pool(name="hat", bufs=3) as hat,
            tc.tile_pool(name="ps_ubc", bufs=2, space="PSUM") as ps_ubc,
            tc.tile_pool(name="ps_a", bufs=2, space="PSUM") as ps_a,
            tc.tile_pool(name="ps_bld", bufs=1, space="PSUM") as ps_bld,
        ):
            # ---- persistent tiles ----
            ones = pers.tile([1, 128], F32R, name="ones")
            tsb = pers.tile([128, NT], F32, name="tsb")
            dts = pers.tile([128, NSTEP], F32, name="dts")
            msk = pers.tile([P2, B2], F32, name="msk")
            bv = pers.tile([128, 1], F32, name="bv")
            u_row = pers.tile([1, NP + 2], F32R, name="u_row")
            u0stg = pers.tile([1, NP + 2], F32, name="u0stg")
            u2e = pers.tile([P2, B2 + 2], F32, name="u2e")
            unw = pers.tile([P2, B2], F32R, name="unw")
            a_row = pers.tile([1, NP], F32, name="a_row")
            a2d = pers.tile([P2, B2], F32, name="a2d")
            tcol = pers.tile([128, 1], F32, name="tcol")
            tbl = pers.tile([128, 1], BF16, name="tbl")
            # flux scratch, [128,17] tiles used as per-half [64,17] slices
            dd = pers.tile([P2, B2], F32, name="dd")
            l1 = pers.tile([P2, B2], F32, name="l1")
            lap = pers.tile([P2, B2], F32, name="lap")
            dtm = pers.tile([P2, B2], F32, name="dtm")
            pP = pers.tile([P2, B2], F32, name="pP")
            qQ = pers.tile([P2, B2], F32, name="qQ")
            tD = pers.tile([P2, B2], F32, name="tD")
            uM = pers.tile([P2, B2], F32, name="uM")
            rR = pers.tile([P2, B2], F32, name="rR")
            aa = pers.tile([P2, B2], F32, name="aa")
            m1 = pers.tile([P2, B2], F32, name="m1")
            m2 = pers.tile([P2, B2], F32, name="m2")
            sm = pers.tile([P2, B2], F32, name="sm")
            zz = pers.tile([P2, B2], F32, name="zz")
            cD = pers.tile([P2, B2], F32, name="cD")
            nga = pers.tile([P2, B2], F32, name="nga")

            # overlapping-window view of u_row: [128 x 19] windows, stride 17
            row_ap = u_row[0:1, 0 : NP + 2].bitcast(F32)
            win_ap = dataclasses.replace(
                row_ap, ap=[list(row_ap.ap[0]), [B2, P2], [1, B2 + 2]]
            )

            # ---- init ----
            ones_f = pers.tile([1, 128], F32, name="ones_f")
            nc.vector.memset(ones_f[:, :], 1.0)
            nc.vector.memset(zz[:, :], 0.0)
            nc.vector.memset(cD[:, :], 2.0 * DX * D_COEF)
            nc.vector.tensor_copy(ones[:, :], ones_f[:, :])
            nc.sync.dma_start(out=tsb[:, :], in_=tbd.ap())
            nc.vector.tensor_sub(dts[:, :], tsb[:, 1:NT], tsb[:, 0 : NT - 1])
            nc.sync.dma_start(out=msk[:, :], in_=mkd.ap())
            nc.sync.dma_start(out=bv[:, :], in_=bvd.ap())
            nc.sync.dma_start(out=u0stg[:, :], in_=u0g.ap())
            nc.vector.tensor_copy(u_row[:, :], u0stg[:, :])
            nc.sync.dma_start(out=u2e[:, :], in_=win_ap)
            # step 0 output = u0
            nc.sync.dma_start(
                out=outd.ap()[0:1, :],
                in_=u0stg[0:1, 1 + GH : 1 + GH + OWN].bitcast(F32),
            )

            # ---- build the PWL table: exact MLP at the K knot positions ----
            w2sb = [bld.tile([128, H], F32R, name=f"w2sb{k}") for k in range(4)]
            w2f = [bld.tile([128, H], F32, name=f"w2f{k}") for k in range(4)]
            w1t = bld.tile([128, 4], F32, name="w1t")
            w3f = bld.tile([128, 4], F32, name="w3f")
            w3t = bld.tile([128, 4], F32R, name="w3t")
            knsb = bld.tile([1, K], F32, name="knsb")
            knr = bld.tile([1, K], F32R, name="knr")
            h1b = [bld.tile([128, K], F32R, name=f"h1b{k}") for k in range(4)]
            h2b = [bld.tile([128, K], F32R, name=f"h2b{k}") for k in range(4)]
            trow = bld.tile([1, K], F32, name="trow")

            for k in range(4):
                nc.sync.dma_start(
                    out=w2f[k][:, :], in_=w2d.ap()[128 * k : 128 * (k + 1), :]
                )
                nc.vector.tensor_copy(w2sb[k][:, :], w2f[k][:, :])
            nc.sync.dma_start(
                out=w1t[:, :], in_=w1d.ap().rearrange("a (c p) -> p (a c)", p=128)
            )
            nc.sync.dma_start(
                out=w3f[:, :], in_=w3d.ap().rearrange("(c p) a -> p (c a)", p=128)
            )
            nc.vector.tensor_copy(w3t[:, :], w3f[:, :])
            nc.sync.dma_start(out=knsb[:, :], in_=knd.ap())
            nc.vector.tensor_copy(knr[:, :], knsb[:, :])

            ub_ps = ps_bld.tile([128, 512], F32, name="ub_ps")
            nc.tensor.matmul(
                out=ub_ps[:, :K], lhsT=ones[0:1, :], rhs=knr[0:1, :],
                start=True, stop=True,
            )
            for j in range(4):
                nc.scalar.activation(
                    out=h1b[j][:, :], in_=ub_ps[:, :K], func=AF.Tanh,
                    scale=w1t[:, j : j + 1],
                )
            for j in range(4):
                h2_ps = ps_bld.tile([128, 512], F32, name="h2_ps")
                for k in range(4):
                    nc.tensor.matmul(
                        out=h2_ps[:, :K],
                        lhsT=w2sb[k][:, 128 * j : 128 * (j + 1)],
                        rhs=h1b[k][:, :],
                        start=(k == 0), stop=(k == 3),
                    )
                nc.scalar.activation(out=h2b[j][:, :], in_=h2_ps[:, :K], func=AF.Tanh)
            ab_ps = ps_bld.tile([1, 512], F32, name="ab_ps")
            for k in range(4):
                nc.tensor.matmul(
                    out=ab_ps[0:1, :K], lhsT=w3t[:, k : k + 1], rhs=h2b[k][:, :],
                    start=(k == 0), stop=(k == 3),
                )
            nc.scalar.activation(out=trow[0:1, :], in_=ab_ps[0:1, :K], func=AF.Tanh)
            nc.sync.dma_start(out=tcol[:, :], in_=trow[0:1, :])
            nc.vector.tensor_copy(tbl[:, :], tcol[:, :])

            # ---- time steps ----
            for s in [s for _ in range(nrep) for s in range(NSTEP)]:
                for hh in range(2):
                    p0 = 64 * hh              # partition base of this half
                    po = HP * hh              # point offset of this half
                    psl = slice(p0, p0 + 64)

                    # flux precomputation (needs only u2e, overlaps the LUT):
                    # unw = P*a + Q*|a| + R
                    uL = u2e[psl, 0:B2]
                    uC = u2e[psl, 1 : 1 + B2]
                    uR = u2e[psl, 2 : 2 + B2]
                    nc.gpsimd.tensor_sub(dd[psl, :], uL, uR)
                    nc.gpsimd.tensor_add(l1[psl, :], uL, uR)
                    nc.gpsimd.tensor_sub(lap[psl, :], l1[psl, :], uC)
                    nc.gpsimd.tensor_sub(lap[psl, :], lap[psl, :], uC)
                    # dtm = msk*dt/(2*DX); tD = lap*dtm*2*DX*D = qQ*cD
                    nc.vector.tensor_scalar(
                        out=dtm[psl, :], in0=msk[psl, :],
                        scalar1=dts[psl, s : s + 1], scalar2=1.0 / (2 * DX),
                        op0=OP.mult, op1=OP.mult,
                    )
                    nc.gpsimd.tensor_mul(pP[psl, :], dd[psl, :], dtm[psl, :])
                    nc.gpsimd.tensor_mul(qQ[psl, :], lap[psl, :], dtm[psl, :])
                    nc.gpsimd.tensor_mul(tD[psl, :], qQ[psl, :], cD[psl, :])
                    nc.gpsimd.tensor_mul(uM[psl, :], uC, msk[psl, :])
                    nc.gpsimd.tensor_add(rR[psl, :], tD[psl, :], uM[psl, :])

                    # LUT phase: a = PWL(u) for this half's points
                    for ci, (co, n) in enumerate(HCH):
                        o = po + co
                        ubc = ps_ubc.tile([128, 512], F32, name="ubc")
                        nc.tensor.matmul(
                            out=ubc[:, :n], lhsT=ones[0:1, :],
                            rhs=u_row[0:1, 1 + o : 1 + o + n],
                            start=True, stop=True,
                        )
                        t1 = hat.tile([128, 512], BF16, name="t1")
                        nc.scalar.activation(
                            out=t1[:, :n], in_=ubc[:, :n], func=AF.Abs,
                            bias=bv[:, 0:1], scale=1.0 / HSTEP,
                        )
                        m = hat.tile([128, 512], BF16, name="m")
                        nc.vector.tensor_scalar(
                            out=m[:, :n], in0=t1[:, :n], scalar1=-1.0,
                            scalar2=1.0, op0=OP.mult, op1=OP.add,
                        )
                        sw = hat.tile([128, 512], BF16, name="sw")
                        nc.vector.tensor_scalar(
                            out=sw[:, :n], in0=m[:, :n], scalar1=0.0,
                            scalar2=None, op0=OP.max,
                        )
                        aps = ps_a.tile([1, 512], F32, name="aps")
                        nc.tensor.matmul(
                            out=aps[0:1, :n], lhsT=tbl[:, 0:1], rhs=sw[:, :n],
                            start=True, stop=True,
                        )
                        # a chunk PSUM -> SBUF row (split ACT / DVE)
                        if ci == 0:
                            nc.scalar.activation(
                                out=a_row[0:1, o : o + n], in_=aps[0:1, :n],
                                func=AF.Identity,
                            )
                        else:
                            nc.vector.tensor_copy(
                                a_row[0:1, o : o + n], aps[0:1, :n]
                            )

                    # reshape a to 2-D for this half
                    nc.sync.dma_start(
                        out=a2d[psl, :],
                        in_=a_row[0:1, po : po + HP].bitcast(F32),
                    )

                    # update: unw = P*a + Q*|a| + R
                    nc.vector.tensor_sub(nga[psl, :], zz[psl, :], a2d[psl, :])
                    nc.vector.tensor_tensor(
                        out=aa[psl, :], in0=nga[psl, :], in1=a2d[psl, :],
                        op=OP.max,
                    )
                    nc.vector.tensor_mul(m1[psl, :], pP[psl, :], a2d[psl, :])
                    nc.vector.tensor_mul(m2[psl, :], qQ[psl, :], aa[psl, :])
                    nc.vector.tensor_add(sm[psl, :], m1[psl, :], m2[psl, :])
                    nc.vector.tensor_add(unw[psl, :], sm[psl, :], rR[psl, :])

                    # write this half back to the row
                    nc.sync.dma_start(
                        out=u_row[0:1, 1 + po : 1 + po + HP], in_=unw[psl, :]
                    )
                    if hh == 1:
                        for h2 in range(2):
                            q0 = 64 * h2
                            rsl = u_row[
                                0:1, B2 * q0 : B2 * q0 + 64 * B2 + 2
                            ].bitcast(F32)
                            w_ap = dataclasses.replace(
                                rsl, ap=[list(rsl.ap[0]), [B2, 64], [1, B2 + 2]]
                            )
                            nc.sync.dma_start(
                                out=u2e[q0 : q0 + 64, :], in_=w_ap
                            )

                nc.sync.dma_start(
                    out=outd.ap()[s + 1 : s + 2, :],
                    in_=u_row[0:1, 1 + GH : 1 + GH + OWN].bitcast(F32),
                )

    nc.finalize()
    return nc


_NC_CACHE = {}


def _get_nc(nrep=1):
    if nrep not in _NC_CACHE:
        _NC_CACHE[nrep] = _build_nc(nrep)
    return _NC_CACHE[nrep]


def _make_in_maps(t, u0, W1, W2, W3):
    t = np.asarray(t, np.float32)
    u0 = np.asarray(u0, np.float32).reshape(NX)
    W1 = np.ascontiguousarray(np.asarray(W1, np.float32).reshape(1, H))
    W2 = np.ascontiguousarray(np.asarray(W2, np.float32).reshape(H, H))
    W3 = np.ascontiguousarray(np.asarray(W3, np.float32).reshape(H, 1))
    tb = np.ascontiguousarray(np.broadcast_to(t.reshape(1, NT), (128, NT)))
    kn = np.ascontiguousarray(
        (LO + HSTEP * np.arange(K, dtype=np.float32)).reshape(1, K)
    )
    bvec = np.ascontiguousarray(
        (-LO / HSTEP - np.arange(128, dtype=np.float32)).reshape(128, 1)
    )

    padded = np.zeros(NX + 2 * (GH + 1), np.float32)
    padded[GH + 1 : GH + 1 + NX] = u0

    in_maps = []
    for c in range(NCORES):
        slab = np.ascontiguousarray(
            padded[c * OWN : c * OWN + NP + 2].reshape(1, NP + 2)
        )
        gidx = c * OWN - GH + np.arange(NP)
        mask = ((gidx >= 0) & (gidx < NX)).astype(np.float32).reshape(P2, B2)
        in_maps.append(
            {
                "u0g": slab,
                "w1": W1,
                "w2": W2,
                "w3": W3,
                "tb": tb,
                "mask": np.ascontiguousarray(mask),
                "kn": kn,
                "biasv": bvec,
            }
        )
    return in_maps


def _run(t, u0, W1, W2, W3, trace=False):
    nc = _get_nc()
    in_maps = _make_in_maps(t, u0, W1, W2, W3)
    res = run_bass_kernel_spmd(
        nc, in_maps, core_ids=list(range(NCORES)), trace=trace,
        trace_cores=list(range(NCORES)) if trace else None,
    )
    parts = [res.results[c]["out"] for c in range(NCORES)]
    full = np.concatenate(parts, axis=1).reshape(NT, NX, 1).astype(np.float32)
    return full, res


def kernel(t, u0, W1, W2, W3):
    full, _ = _run(t, u0, W1, W2, W3, trace=False)
    return full


# revision 14
# speedup vs baseline: 1.1281x; 1.1281x over previous
"""FINN Burgers solver (nn_FINN_Burger) as a Trainium2 Bass kernel.

The per-point MLP a = tanh(tanh(tanh(u W1) W2) W3) is a scalar function
F: R -> R of the cell value u alone.  F is smooth (max |F''| ~ 1.3), so a
128-knot piecewise-linear interpolant matches it to ~1e-3 -- far inside the
harness tolerance.  The kernel therefore:

  1. builds the table on device: one exact-MLP evaluation at the 128 knot
     positions (same matmul pipeline as the baseline kernel, one-time cost),
  2. per Euler step evaluates a = PWL(u) via a "two-hot" matmul:
        y    = u/h                      (PE ones-broadcast to 128 partitions)
        t1   = |y - c_p|                (ACT Abs, per-partition bias c_p)
        S    = relu(1 - t1)             (DVE, two bf16 4x-mode ops)
        a    = T^T @ S                  (PE matmul, table as weights)
     S holds exactly the two interpolation weights per point, so the matmul
     contraction over the 128 knot partitions IS the interpolation,
  3. computes the flux + Euler + mask update as  unw = P*a + Q*|a| + R  in a
     [128,17] 2-D layout, where P, Q, R depend only on u and are computed
     during the LUT phase (off the critical path):
        flux = D*lap + (dd*a + lap*|a|) / (2*DX),  dd = u_l - u_r,
        lap  = u_l + u_r - 2u   (relu(a) = (a+|a|)/2, min(a,0) = (a-|a|)/2)

The slab is processed as two 1088-point halves so the end-of-step reshape
DMAs of one half overlap the LUT phase of the other half / the next step.

Sharding: Nx=16384 split across 8 cores (2048 points each) with a 64-point
ghost zone per side; 15 steps need only a 15-point halo, so each core
integrates its 2176-point slab fully locally -- zero inter-core traffic.
Out-of-domain ghost points are zeroed each step via the mask (also the
Dirichlet boundary for cores 0 and 7).
"""

import dataclasses

import numpy as np

import concourse.bacc as bacc
import concourse.bass as bass
import concourse.mybir as mybir
from concourse import tile
from concourse.bass_utils import run_bass_kernel_spmd

F32 = mybir.dt.float32
F32R = mybir.dt.float32r
BF16 = mybir.dt.bfloat16
AF = mybir.ActivationFunctionType
OP = mybir.AluOpType

NX, H, NT = 16384, 512, 16
NCORES = 8
OWN = NX // NCORES          # 2048 points owned per core
P2, B2 = 128, 17            # 2-D layout: partition p holds points [17p,17p+17)
NP = P2 * B2                # 2176-point slab
GH = (NP - OWN) // 2        # 64-point ghost zone per side (need >= 15)
NSTEP = NT - 1
DX = 0.01
D_COEF = 0.01

K = 128                     # PWL knots
LO, HI = -5.5, 5.5
HSTEP = (HI - LO) / (K - 1)
HP = NP // 2                # 1088 points per half
# per-half point chunks (offsets relative to half start)
HCH = [(0, 512), (512, 512), (1024, 64)]


def _build_nc(nrep=1):
    nc = bacc.Bacc("TRN2", target_bir_lowering=False, debug=False)

    u0g = nc.dram_tensor("u0g", [1, NP + 2], F32, kind="ExternalInput")
    w1d = nc.dram_tensor("w1", [1, H], F32, kind="ExternalInput")
    w2d = nc.dram_tensor("w2", [H, H], F32, kind="ExternalInput")
    w3d = nc.dram_tensor("w3", [H, 1], F32, kind="ExternalInput")
    tbd = nc.dram_tensor("tb", [128, NT], F32, kind="ExternalInput")
    mkd = nc.dram_tensor("mask", [P2, B2], F32, kind="ExternalInput")
    knd = nc.dram_tensor("kn", [1, K], F32, kind="ExternalInput")
    bvd = nc.dram_tensor("biasv", [128, 1], F32, kind="ExternalInput")
    outd = nc.dram_tensor("out", [NT, OWN], F32, kind="ExternalOutput")

    with tile.TileContext(nc) as tc:
        with (
            tc.tile_pool(name="pers", bufs=1) as pers,
            tc.tile_pool(name="bld", bufs=1) as bld,
            tc.tile_pool(name="hat", bufs=3) as hat,
            tc.tile_pool(name="ps_ubc", bufs=2, space="PSUM") as ps_ubc,
            tc.tile_pool(name="ps_a", bufs=2, space="PSUM") as ps_a,
            tc.tile_pool(name="ps_bld", bufs=1, space="PSUM") as ps_bld,
        ):
            # ---- persistent tiles ----
            ones = pers.tile([1, 128], F32R, name="ones")
            tsb = pers.tile([128, NT], F32, name="tsb")
            dts = pers.tile([128, NSTEP], F32, name="dts")
            msk = pers.tile([P2, B2], F32, name="msk")
            bv = pers.tile([128, 1], F32, name="bv")
            u_row = pers.tile([1, NP + 2], F32R, name="u_row")
            u0stg = pers.tile([1, NP + 2], F32, name="u0stg")
            u2e = pers.tile([P2, B2 + 2], F32, name="u2e")
            unw = pers.tile([P2, B2], F32R, name="unw")
            a_row = pers.tile([1, NP], F32, name="a_row")
            a2d = pers.tile([P2, B2], F32, name="a2d")
            tcol = pers.tile([128, 1], F32, name="tcol")
            tbl = pers.tile([128, 1], BF16, name="tbl")
            # flux scratch, [128,17] tiles used as per-half [64,17] slices
            dd = pers.tile([P2, B2], F32, name="dd")
            l1 = pers.tile([P2, B2], F32, name="l1")
            lap = pers.tile([P2, B2], F32, name="lap")
            dtm = pers.tile([P2, B2], F32, name="dtm")
            pP = pers.tile([P2, B2], F32, name="pP")
            qQ = pers.tile([P2, B2], F32, name="qQ")
            tD = pers.tile([P2, B2], F32, name="tD")
            uM = pers.tile([P2, B2], F32, name="uM")
            rR = pers.tile([P2, B2], F32, name="rR")
            aa = pers.tile([P2, B2], F32, name="aa")
            m1 = pers.tile([P2, B2], F32, name="m1")
            m2 = pers.tile([P2, B2], F32, name="m2")
            sm = pers.tile([P2, B2], F32, name="sm")
            zz = pers.tile([P2, B2], F32, name="zz")
            cD = pers.tile([P2, B2], F32, name="cD")
            nga = pers.tile([P2, B2], F32, name="nga")

            # overlapping-window view of u_row: [128 x 19] windows, stride 17
            row_ap = u_row[0:1, 0 : NP + 2].bitcast(F32)
            win_ap = dataclasses.replace(
                row_ap, ap=[list(row_ap.ap[0]), [B2, P2], [1, B2 + 2]]
            )

            # ---- init ----
            ones_f = pers.tile([1, 128], F32, name="ones_f")
            nc.vector.memset(ones_f[:, :], 1.0)
            nc.vector.memset(zz[:, :], 0.0)
            nc.vector.memset(cD[:, :], 2.0 * DX * D_COEF)
            nc.vector.tensor_copy(ones[:, :], ones_f[:, :])
            nc.sync.dma_start(out=tsb[:, :], in_=tbd.ap())
            nc.vector.tensor_sub(dts[:, :], tsb[:, 1:NT], tsb[:, 0 : NT - 1])
            nc.sync.dma_start(out=msk[:, :], in_=mkd.ap())
            nc.sync.dma_start(out=bv[:, :], in_=bvd.ap())
            nc.sync.dma_start(out=u0stg[:, :], in_=u0g.ap())
            nc.vector.tensor_copy(u_row[:, :], u0stg[:, :])
            nc.sync.dma_start(out=u2e[:, :], in_=win_ap)
            # step 0 output = u0
            nc.sync.dma_start(
                out=outd.ap()[0:1, :],
                in_=u0stg[0:1, 1 + GH : 1 + GH + OWN].bitcast(F32),
            )

            # ---- build the PWL table: exact MLP at the K knot positions ----
            w2sb = [bld.tile([128, H], F32R, name=f"w2sb{k}") for k in range(4)]
            w2f = [bld.tile([128, H], F32, name=f"w2f{k}") for k in range(4)]
            w1t = bld.tile([128, 4], F32, name="w1t")
            w3f = bld.tile([128, 4], F32, name="w3f")
            w3t = bld.tile([128, 4], F32R, name="w3t")
            knsb = bld.tile([1, K], F32, name="knsb")
            knr = bld.tile([1, K], F32R, name="knr")
            h1b = [bld.tile([128, K], F32R, name=f"h1b{k}") for k in range(4)]
            h2b = [bld.tile([128, K], F32R, name=f"h2b{k}") for k in range(4)]
            trow = bld.tile([1, K], F32, name="trow")

            for k in range(4):
                nc.sync.dma_start(
                    out=w2f[k][:, :], in_=w2d.ap()[128 * k : 128 * (k + 1), :]
                )
                nc.vector.tensor_copy(w2sb[k][:, :], w2f[k][:, :])
            nc.sync.dma_start(
                out=w1t[:, :], in_=w1d.ap().rearrange("a (c p) -> p (a c)", p=128)
            )
            nc.sync.dma_start(
                out=w3f[:, :], in_=w3d.ap().rearrange("(c p) a -> p (c a)", p=128)
            )
            nc.vector.tensor_copy(w3t[:, :], w3f[:, :])
            nc.sync.dma_start(out=knsb[:, :], in_=knd.ap())
            nc.vector.tensor_copy(knr[:, :], knsb[:, :])

            ub_ps = ps_bld.tile([128, 512], F32, name="ub_ps")
            nc.tensor.matmul(
                out=ub_ps[:, :K], lhsT=ones[0:1, :], rhs=knr[0:1, :],
                start=True, stop=True,
            )
            for j in range(4):
                nc.scalar.activation(
                    out=h1b[j][:, :], in_=ub_ps[:, :K], func=AF.Tanh,
                    scale=w1t[:, j : j + 1],
                )
            for j in range(4):
                h2_ps = ps_bld.tile([128, 512], F32, name="h2_ps")
                for k in range(4):
                    nc.tensor.matmul(
                        out=h2_ps[:, :K],
                        lhsT=w2sb[k][:, 128 * j : 128 * (j + 1)],
                        rhs=h1b[k][:, :],
                        start=(k == 0), stop=(k == 3),
                    )
                nc.scalar.activation(out=h2b[j][:, :], in_=h2_ps[:, :K], func=AF.Tanh)
            ab_ps = ps_bld.tile([1, 512], F32, name="ab_ps")
            for k in range(4):
                nc.tensor.matmul(
                    out=ab_ps[0:1, :K], lhsT=w3t[:, k : k + 1], rhs=h2b[k][:, :],
                    start=(k == 0), stop=(k == 3),
                )
            nc.scalar.activation(out=trow[0:1, :], in_=ab_ps[0:1, :K], func=AF.Tanh)
            nc.sync.dma_start(out=tcol[:, :], in_=trow[0:1, :])
            nc.vector.tensor_copy(tbl[:, :], tcol[:, :])

            # ---- time steps ----
            for s in [s for _ in range(nrep) for s in range(NSTEP)]:
                for hh in range(2):
                    p0 = 64 * hh              # partition base of this half
                    po = HP * hh              # point offset of this half
                    psl = slice(p0, p0 + 64)

                    # flux precomputation (needs only u2e, overlaps the LUT):
                    # unw = P*a + Q*|a| + R
                    uL = u2e[psl, 0:B2]
                    uC = u2e[psl, 1 : 1 + B2]
                    uR = u2e[psl, 2 : 2 + B2]
                    nc.gpsimd.tensor_sub(dd[psl, :], uL, uR)
                    nc.gpsimd.tensor_add(l1[psl, :], uL, uR)
                    nc.gpsimd.tensor_sub(lap[psl, :], l1[psl, :], uC)
                    nc.gpsimd.tensor_sub(lap[psl, :], lap[psl, :], uC)
                    # dtm = msk*dt/(2*DX); tD = lap*dtm*2*DX*D = qQ*cD
                    nc.vector.tensor_scalar(
                        out=dtm[psl, :], in0=msk[psl, :],
                        scalar1=dts[psl, s : s + 1], scalar2=1.0 / (2 * DX),
                        op0=OP.mult, op1=OP.mult,
                    )
                    nc.gpsimd.tensor_mul(pP[psl, :], dd[psl, :], dtm[psl, :])
                    nc.gpsimd.tensor_mul(qQ[psl, :], lap[psl, :], dtm[psl, :])
                    nc.gpsimd.tensor_mul(tD[psl, :], qQ[psl, :], cD[psl, :])
                    nc.gpsimd.tensor_mul(uM[psl, :], uC, msk[psl, :])
                    nc.gpsimd.tensor_add(rR[psl, :], tD[psl, :], uM[psl, :])

                    # LUT phase: a = PWL(u) for this half's points
                    for ci, (co, n) in enumerate(HCH):
                        o = po + co
                        ubc = ps_ubc.tile([128, 512], F32, name="ubc")
                        nc.tensor.matmul(
                            out=ubc[:, :n], lhsT=ones[0:1, :],
                            rhs=u_row[0:1, 1 + o : 1 + o + n],
                            start=True, stop=True,
                        )
                        t1 = hat.tile([128, 512], BF16, name="t1")
                        nc.scalar.activation(
                            out=t1[:, :n], in_=ubc[:, :n], func=AF.Abs,
                            bias=bv[:, 0:1], scale=1.0 / HSTEP,
                        )
                        m = hat.tile([128, 512], BF16, name="m")
                        nc.vector.tensor_scalar(
                            out=m[:, :n], in0=t1[:, :n], scalar1=-1.0,
                            scalar2=1.0, op0=OP.mult, op1=OP.add,
                        )
                        sw = hat.tile([128, 512], BF16, name="sw")
                        nc.vector.tensor_scalar(
                            out=sw[:, :n], in0=m[:, :n], scalar1=0.0,
                            scalar2=None, op0=OP.max,
                        )
                        aps = ps_a.tile([1, 512], F32, name="aps")
                        nc.tensor.matmul(
                            out=aps[0:1, :n], lhsT=tbl[:, 0:1], rhs=sw[:, :n],
                            start=True, stop=True,
                        )
                        # a chunk PSUM -> SBUF row (split ACT / DVE)
                        if ci == 0:
                            nc.scalar.activation(
                                out=a_row[0:1, o : o + n], in_=aps[0:1, :n],
                                func=AF.Identity,
                            )
                        else:
                            nc.vector.tensor_copy(
                                a_row[0:1, o : o + n], aps[0:1, :n]
                            )

                    # reshape a to 2-D for this half
                    nc.sync.dma_start(
                        out=a2d[psl, :],
                        in_=a_row[0:1, po : po + HP].bitcast(F32),
                    )

                    # update: unw = P*a + Q*|a| + R
                    nc.gpsimd.tensor_sub(nga[psl, :], zz[psl, :], a2d[psl, :])
                    nc.vector.tensor_tensor(
                        out=aa[psl, :], in0=nga[psl, :], in1=a2d[psl, :],
                        op=OP.max,
                    )
                    nc.gpsimd.tensor_mul(m1[psl, :], pP[psl, :], a2d[psl, :])
                    nc.vector.tensor_mul(m2[psl, :], qQ[psl, :], aa[psl, :])
                    nc.gpsimd.tensor_add(sm[psl, :], m1[psl, :], m2[psl, :])
                    nc.vector.tensor_add(unw[psl, :], sm[psl, :], rR[psl, :])

                    # write this half back to the row
                    nc.sync.dma_start(
                        out=u_row[0:1, 1 + po : 1 + po + HP], in_=unw[psl, :]
                    )
                    if hh == 1:
                        for h2 in range(2):
                            q0 = 64 * h2
                            rsl = u_row[
                                0:1, B2 * q0 : B2 * q0 + 64 * B2 + 2
                            ].bitcast(F32)
                            w_ap = dataclasses.replace(
                                rsl, ap=[list(rsl.ap[0]), [B2, 64], [1, B2 + 2]]
                            )
                            nc.sync.dma_start(
                                out=u2e[q0 : q0 + 64, :], in_=w_ap
                            )

                nc.sync.dma_start(
                    out=outd.ap()[s + 1 : s + 2, :],
                    in_=u_row[0:1, 1 + GH : 1 + GH + OWN].bitcast(F32),
                )

    nc.finalize()
    return nc


_NC_CACHE = {}


def _get_nc(nrep=1):
    if nrep not in _NC_CACHE:
        _NC_CACHE[nrep] = _build_nc(nrep)
    return _NC_CACHE[nrep]


def _make_in_maps(t, u0, W1, W2, W3):
    t = np.asarray(t, np.float32)
    u0 = np.asarray(u0, np.float32).reshape(NX)
    W1 = np.ascontiguousarray(np.asarray(W1, np.float32).reshape(1, H))
    W2 = np.ascontiguousarray(np.asarray(W2, np.float32).reshape(H, H))
    W3 = np.ascontiguousarray(np.asarray(W3, np.float32).reshape(H, 1))
    tb = np.ascontiguousarray(np.broadcast_to(t.reshape(1, NT), (128, NT)))
    kn = np.ascontiguousarray(
        (LO + HSTEP * np.arange(K, dtype=np.float32)).reshape(1, K)
    )
    bvec = np.ascontiguousarray(
        (-LO / HSTEP - np.arange(128, dtype=np.float32)).reshape(128, 1)
    )

    padded = np.zeros(NX + 2 * (GH + 1), np.float32)
    padded[GH + 1 : GH + 1 + NX] = u0

    in_maps = []
    for c in range(NCORES):
        slab = np.ascontiguousarray(
            padded[c * OWN : c * OWN + NP + 2].reshape(1, NP + 2)
        )
        gidx = c * OWN - GH + np.arange(NP)
        mask = ((gidx >= 0) & (gidx < NX)).astype(np.float32).reshape(P2, B2)
        in_maps.append(
            {
                "u0g": slab,
                "w1": W1,
                "w2": W2,
                "w3": W3,
                "tb": tb,
                "mask": np.ascontiguousarray(mask),
                "kn": kn,
                "biasv": bvec,
            }
        )
    return in_maps


def _run(t, u0, W1, W2, W3, trace=False):
    nc = _get_nc()
    in_maps = _make_in_maps(t, u0, W1, W2, W3)
    res = run_bass_kernel_spmd(
        nc, in_maps, core_ids=list(range(NCORES)), trace=trace,
        trace_cores=list(range(NCORES)) if trace else None,
    )
    parts = [res.results[c]["out"] for c in range(NCORES)]
    full = np.concatenate(parts, axis=1).reshape(NT, NX, 1).astype(np.float32)
    return full, res


def kernel(t, u0, W1, W2, W3):
    full, _ = _run(t, u0, W1, W2, W3, trace=False)
    return full
